# revision 1
# baseline (speedup 1.0000x reference)
"""Trainium2 Bass kernel: full encoder-decoder transformer decoder layer.

Contract: kernel(**inputs) takes FULL unsharded inputs (see below) and
returns the FULL [B, T, D] float32 output.

Sharding: pure data-parallel over (batch, T-half) -> 8 cores, zero
collectives.  Each core computes its TL=1024 decoder rows end-to-end;
the full-T K/V projections are computed redundantly by the 2 cores
sharing a batch element (~17% extra FLOPs, no cross-core sync).

On-device layout: the token stream is carried TRANSPOSED [D, T] so that
every projection matmul uses a natural weight slice as the stationary
(lhsT) operand and outputs stay transposed.  Attention computes
S^T = K @ Q^T per head, exp on ScalarE (scale=1/sqrt(hd) fused), then
O^T via V-stationary accumulation where an appended ones column yields
the softmax denominators in the same PSUM tile.  LayerNorm reduces over
D (the partition dim) with a ones[128,128] matmul that also broadcasts
the stats to all partitions.
"""

from contextlib import ExitStack

import ml_dtypes
import numpy as np

import concourse.bass as bass
import concourse.mybir as mybir
import concourse.tile as tile
from concourse import bacc
from concourse.bass_utils import run_bass_kernel_spmd

P = 128
HD = 64  # head dim (fixed)
BF = mybir.dt.bfloat16
F32 = mybir.dt.float32
AF = mybir.ActivationFunctionType
ALU = mybir.AluOpType
EPS = 1e-5


# ----------------------------------------------------------------------------
# device program builder
# ----------------------------------------------------------------------------

def build_program(D=1024, H=16, T=2048, TL=1024, S=2048, DFF=4096, loop_n=1):
    """Build the single-core SPMD Bass program.

    D: model dim; H: heads; T: full decoder length (K/V span);
    TL: local query rows; S: encoder length; DFF: ffn dim.
    """
    assert D == H * HD
    KT = D // P            # D tiles
    NKT = T // P           # self-attn k tiles
    NSK = S // P           # cross-attn k tiles
    FT = DFF // P          # ffn tiles
    HP = H // 2            # packed head-pair tiles (== KT)
    assert HP == KT
    QC = min(512, TL)      # query chunk (psum free dim)
    NQ = TL // QC

    nc = bacc.Bacc()

    tens = {}

    def din(name, shape, dtype=BF):
        tens[name] = nc.declare_dram_parameter(name, list(shape), dtype,
                                               isOutput=False)
        return tens[name]

    # streams
    xdT = din("xdT", (KT, P, T))          # decoder stream, transposed, full T
    xqT = din("xqT", (KT, P, TL))         # own-query slice (bf16)
    xres = din("xres", (KT, P, TL), F32)  # own residual slice (f32)
    xeT = din("xeT", (KT, P, S))          # encoder stream, transposed
    # weights (column-permuted on host into head-major [h*64+d] order)
    for nm in ("wq", "wk", "wv", "wo1", "wqc", "wkc", "wvc", "wo2"):
        din(nm, (KT, P, D))
    din("w1", (KT, P, DFF))
    din("w2", (FT, P, D))
    # per-partition bias / LN params (f32)
    for nm in ("bq", "bk", "bo1", "bqc", "bkc", "bo2", "b2f",
               "g1", "be1", "g2", "be2", "g3", "be3"):
        din(nm, (KT, P, 1), F32)
    din("b1f", (FT, P, 1), F32)
    # free-dim bias rows (for V projections, broadcast via DMA)
    din("bv_row", (1, D), F32)
    din("bvc_row", (1, D), F32)

    tens["outT"] = nc.declare_dram_parameter("outT", [KT, P, TL], F32,
                                             isOutput=True)

    # internal DRAM spill for the two residual streams
    tens["res1_spill"] = nc.dram_tensor("res1_spill", [KT, P, TL], F32)
    tens["res2_spill"] = nc.dram_tensor("res2_spill", [KT, P, TL], F32)
    # bounce buffer for softmax reciprocal rows (DRAM-source partition bcast)
    tens["r_bounce"] = nc.dram_tensor("r_bounce", [2, H, NQ, QC], F32)

    cfg = dict(D=D, H=H, T=T, TL=TL, S=S, DFF=DFF, KT=KT, NKT=NKT,
               NSK=NSK, FT=FT, HP=HP, QC=QC, NQ=NQ, tens=tens)

    with tile.TileContext(nc) as tc:
        if loop_n > 1:
            with tc.For_i(0, loop_n, 1) as _i:
                _build(tc, cfg)
        else:
            _build(tc, cfg)

    nc.finalize()
    return nc


def _build(tc, cfg):
    nc = tc.nc
    D, H, T, TL, S, DFF = (cfg["D"], cfg["H"], cfg["T"], cfg["TL"], cfg["S"],
                           cfg["DFF"])
    KT, NKT, NSK, FT, HP, QC, NQ = (cfg["KT"], cfg["NKT"], cfg["NSK"],
                                    cfg["FT"], cfg["HP"], cfg["QC"], cfg["NQ"])

    tens = cfg["tens"]

    def dram(name):
        return tens[name][:]

    ctx = ExitStack()
    # ------------- global pools (never closed before ctx exit) -------------
    const = ctx.enter_context(tc.tile_pool(name="const", bufs=1))
    stream2 = ctx.enter_context(tc.tile_pool(name="stream2", bufs=2))
    # psum: acc [128,512] x4 banks + scores [128,2*QC] x2bufs (4 banks)
    acc = ctx.enter_context(tc.tile_pool(name="acc", bufs=4, space="PSUM"))
    scp = ctx.enter_context(tc.tile_pool(name="scp", bufs=2, space="PSUM"))

    # ---------------- constants ----------------
    ones_bf = const.tile([P, P], BF, tag="ones_bf", name="ones_bf")
    nc.vector.memset(ones_bf[:], 1.0)
    eps_t = const.tile([P, 1], F32, tag="eps_t", name="eps_t")
    nc.vector.memset(eps_t[:], EPS)

    def load_pp(name, n):
        out = []
        src = dram(name)
        for j in range(n):
            tl_ = const.tile([P, 1], F32, tag=f"{name}{j}", name=f"{name}{j}")
            nc.sync.dma_start(out=tl_[:], in_=src[j])
            out.append(tl_)
        return out

    bq = load_pp("bq", KT); bk = load_pp("bk", KT)
    bo1 = load_pp("bo1", KT); bqc = load_pp("bqc", KT)
    bkc = load_pp("bkc", KT); bo2 = load_pp("bo2", KT)
    b1f = load_pp("b1f", FT); b2f = load_pp("b2f", KT)
    g1 = load_pp("g1", KT); be1 = load_pp("be1", KT)
    g2 = load_pp("g2", KT); be2 = load_pp("be2", KT)
    g3 = load_pp("g3", KT); be3 = load_pp("be3", KT)

    def bias_bcast(name):
        tl_ = const.tile([P, D], F32, tag=f"{name}_bc", name=f"{name}_bc")
        src = dram(name)
        bcast_ap = bass.AP(tensor=src.tensor, offset=0, ap=[[0, P], [1, D]])
        nc.gpsimd.dma_start(out=tl_[:], in_=bcast_ap)
        return tl_

    vb_bc = bias_bcast("bv_row")
    vcb_bc = bias_bcast("bvc_row")

    # =====================================================================
    # generic transposed projection:  out^T[F, t] = sum_ki W[ki]^T @ x[ki]
    # fg-outer so each weight chunk is DMA'd exactly once; scoped w pool.
    # =====================================================================
    ACCG = 2

    def projT(wname, x_tiles, nF, Tlen, evict, kt_in=None):
        kt_in = kt_in if kt_in is not None else KT
        C = min(512, Tlen)
        ntc = Tlen // C
        nfj = nF // P
        w = dram(wname)
        with tc.tile_pool(name=f"wp_{wname}", bufs=2) as wp:
            for fg in range((nfj + ACCG - 1) // ACCG):
                js = list(range(fg * ACCG, min((fg + 1) * ACCG, nfj)))
                wts = []
                for ki in range(kt_in):
                    wt = wp.tile([P, len(js) * P], BF, tag=f"k{ki}",
                                 name=f"w_{wname}_{ki}")
                    nc.sync.dma_start(
                        out=wt[:], in_=w[ki][:, js[0] * P:(js[-1] + 1) * P])
                    wts.append(wt)
                for tci in range(ntc):
                    ps = [acc.tile([P, C], F32, tag="acc", name="acc_ps")
                          for _ in js]
                    for ki in range(kt_in):
                        for jj in range(len(js)):
                            nc.tensor.matmul(
                                ps[jj][:],
                                lhsT=wts[ki][:, jj * P:(jj + 1) * P],
                                rhs=x_tiles[ki][:, tci * C:(tci + 1) * C],
                                start=(ki == 0), stop=(ki == kt_in - 1))
                    for jj, fj in enumerate(js):
                        evict(fj, tci, C, ps[jj])

    # =====================================================================
    # attention (self- and cross-)
    # =====================================================================
    def attention(KTt, QTt, Vst, nkt, mergedT, expp, tag, bounce_idx=0,
                  filler=None):
        # Head pairs interleave: even head streams through PE rows 0-63,
        # odd head through rows 64-127 (concurrent row groups).
        rb = tens["r_bounce"]
        with tc.tile_pool(name=f"sm_{tag}", bufs=4) as small:
            for jt in range(HP):
                for qi in range(NQ):
                    avE = acc.tile([P, QC], F32, tag="acc", name="avE")
                    avO = acc.tile([P, QC], F32, tag="acc", name="avO")
                    qsl = slice(qi * QC, (qi + 1) * QC)
                    for kt in range(nkt):
                        sc = scp.tile([P, 2 * QC], F32, tag="sc", name="sc_ps")
                        ksl = slice(kt * P, (kt + 1) * P)
                        nc.tensor.matmul(
                            sc[:, 0:QC],
                            lhsT=KTt[jt][0:HD, ksl], rhs=QTt[jt][0:HD, qsl],
                            start=True, stop=True, tile_position=(0, 0))
                        nc.tensor.matmul(
                            sc[:, QC:2 * QC],
                            lhsT=KTt[jt][HD:P, ksl], rhs=QTt[jt][HD:P, qsl],
                            start=True, stop=True, tile_position=(HD, 0))
                        ex = expp.tile([P, 2 * QC], BF, tag="ex",
                                       name=f"ex_{tag}")
                        nc.scalar.activation(ex[:], sc[:], AF.Exp, scale=0.125)
                        vsl = Vst[kt][:].rearrange("p (h c) -> p h c", c=HD + 1)
                        nc.tensor.matmul(
                            avE[0:HD + 1, :], lhsT=vsl[:, 2 * jt, :],
                            rhs=ex[:, 0:QC],
                            start=(kt == 0), stop=(kt == nkt - 1))
                        nc.tensor.matmul(
                            avO[0:HD + 1, :], lhsT=vsl[:, 2 * jt + 1, :],
                            rhs=ex[:, QC:2 * QC],
                            start=(kt == 0), stop=(kt == nkt - 1))
                    for av, hh in ((avE, 2 * jt), (avO, 2 * jt + 1)):
                        b0 = (hh % 2) * HD
                        r = small.tile([1, QC], F32, tag="recip", name="recip")
                        nc.vector.reciprocal(r[:], av[HD:HD + 1, :])
                        rslot = rb[bounce_idx, hh, qi]
                        nc.sync.dma_start(out=rslot, in_=r[:])
                        bc = small.tile([HD, QC], F32, tag="bcast",
                                        name="bcast")
                        r_bcast = bass.AP(tensor=rslot.tensor,
                                          offset=rslot.offset,
                                          ap=[[0, HD]] + list(rslot.ap[-1:]))
                        nc.gpsimd.dma_start(out=bc[:], in_=r_bcast)
                        nc.vector.tensor_mul(
                            mergedT[jt][b0:b0 + HD, qsl], av[0:HD, :], bc[:])

    # =====================================================================
    # layernorm over D (partition dim), transposed layout.
    # y (f32) overwrites z_f32 in place; optional bf16 copy / spill / output.
    # =====================================================================
    def layer_norm(z_f32, z_bf, g, be, out_bf=None, spill=None, out_dram=None):
        inv_d = 1.0 / D
        with tc.tile_pool(name="lnp", bufs=2) as lnp:
            for tci in range(NQ):
                sl = slice(tci * QC, (tci + 1) * QC)
                psA = acc.tile([P, QC], F32, tag="acc", name="psA")
                psB = acc.tile([P, QC], F32, tag="acc", name="psB")
                for ki in range(KT):
                    nc.tensor.matmul(psA[:], lhsT=ones_bf[:],
                                     rhs=z_bf[ki][:, sl],
                                     start=(ki == 0), stop=(ki == KT - 1))
                zsq = []
                for ki in range(KT):
                    zq = lnp.tile([P, QC], BF, tag=f"zsq{ki % 4}",
                                  name="zsq")
                    nc.vector.tensor_mul(zq[:], z_bf[ki][:, sl],
                                         z_bf[ki][:, sl])
                    zsq.append(zq)
                for ki in range(KT):
                    nc.tensor.matmul(psB[:], lhsT=ones_bf[:], rhs=zsq[ki][:],
                                     start=(ki == 0), stop=(ki == KT - 1))
                mean = lnp.tile([P, QC], F32, tag="mean", name="mean")
                msq = lnp.tile([P, QC], F32, tag="msq", name="msq")
                nc.vector.tensor_scalar_mul(mean[:], psA[:], inv_d)
                nc.vector.tensor_scalar_mul(msq[:], psB[:], inv_d)
                var = lnp.tile([P, QC], F32, tag="var", name="var")
                nc.vector.tensor_mul(var[:], mean[:], mean[:])
                nc.vector.tensor_sub(var[:], msq[:], var[:])
                sd = lnp.tile([P, QC], F32, tag="sd", name="sd")
                nc.scalar.activation(sd[:], var[:], AF.Sqrt, bias=eps_t[:])
                rstd = lnp.tile([P, QC], F32, tag="rstd", name="rstd")
                nc.vector.reciprocal(rstd[:], sd[:])
                mr = lnp.tile([P, QC], F32, tag="mr", name="mr")
                nc.vector.tensor_mul(mr[:], mean[:], rstd[:])
                for ki in range(KT):
                    tmp = lnp.tile([P, QC], F32, tag="lntmp", name="lntmp")
                    nc.vector.tensor_mul(tmp[:], z_f32[ki][:, sl], rstd[:])
                    nc.vector.tensor_sub(tmp[:], tmp[:], mr[:])
                    nc.vector.tensor_scalar(
                        out=z_f32[ki][:, sl], in0=tmp[:],
                        scalar1=g[ki][:], scalar2=be[ki][:],
                        op0=ALU.mult, op1=ALU.add)
                    if out_bf is not None:
                        nc.vector.tensor_copy(out=out_bf[ki][:, sl],
                                              in_=z_f32[ki][:, sl])
                    if spill is not None:
                        nc.sync.dma_start(out=spill[ki][:, sl],
                                          in_=z_f32[ki][:, sl])
                    if out_dram is not None:
                        nc.sync.dma_start(out=out_dram[ki][:, sl],
                                          in_=z_f32[ki][:, sl])

    NVJ = D // 512 if D >= 512 else 1
    VC = min(512, D)

    def v_proj(x_tiles, wv_tiles, Vst, nkt, vbias_bc):
        for kt in range(nkt):
            vsl = Vst[kt][:].rearrange("p (h c) -> p h c", c=HD + 1)
            nc.vector.memset(vsl[:, :, HD:HD + 1], 1.0)
            for vj in range(NVJ):
                ps = acc.tile([P, VC], F32, tag="acc", name="v_ps")
                for ki in range(KT):
                    nc.tensor.matmul(
                        ps[:], lhsT=x_tiles[ki][:, kt * P:(kt + 1) * P],
                        rhs=wv_tiles[ki][:, vj * VC:(vj + 1) * VC],
                        start=(ki == 0), stop=(ki == KT - 1))
                nc.vector.tensor_add(
                    vsl[:, (VC // HD) * vj:(VC // HD) * (vj + 1), 0:HD],
                    ps[:], vbias_bc[:, vj * VC:(vj + 1) * VC])

    # =====================================================================
    # phase structure (pool opens/closes must be LIFO per memory space)
    # =====================================================================
    es_pres = ExitStack()
    pres = es_pres.enter_context(tc.tile_pool(name="pres", bufs=1))
    res_bf = [pres.tile([P, TL], BF, tag=f"rb{k}", name=f"rb{k}")
              for k in range(KT)]

    es_mg = ExitStack()
    pmg = es_mg.enter_context(tc.tile_pool(name="pmg", bufs=1))
    mergedT = [pmg.tile([P, TL], BF, tag=f"mg{j}", name=f"mg{j}")
               for j in range(HP)]

    es_kt = ExitStack()
    pkt = es_kt.enter_context(tc.tile_pool(name="pkt", bufs=1))
    KTt = [pkt.tile([P, T], BF, tag=f"KTt{j}", name=f"KTt{j}")
           for j in range(HP)]
    QTt = [pkt.tile([P, TL], BF, tag=f"QTt{j}", name=f"QTt{j}")
           for j in range(HP)]
    Vst = [pkt.tile([P, H * (HD + 1)], BF, tag=f"Vst{k}", name=f"Vst{k}")
           for k in range(NKT)]

    # ---- Phase 1: self QKV ----
    es_x = ExitStack()
    px = es_x.enter_context(tc.tile_pool(name="px", bufs=1))
    xdT, xqT, vw = [], [], []
    for ki in range(KT):
        tl_ = px.tile([P, T], BF, tag=f"xdT{ki}", name=f"xdT{ki}")
        nc.sync.dma_start(out=tl_[:], in_=dram("xdT")[ki])
        xdT.append(tl_)
    for ki in range(KT):
        tl_ = px.tile([P, TL], BF, tag=f"xqT{ki}", name=f"xqT{ki}")
        nc.sync.dma_start(out=tl_[:], in_=dram("xqT")[ki])
        xqT.append(tl_)
    for ki in range(KT):
        tl_ = px.tile([P, D], BF, tag=f"vw{ki}", name=f"vw{ki}")
        nc.sync.dma_start(out=tl_[:], in_=dram("wv")[ki])
        vw.append(tl_)

    def ev_k(fj, tci, C, ps):
        nc.vector.tensor_scalar_add(
            out=KTt[fj][:, tci * C:(tci + 1) * C], in0=ps[:], scalar1=bk[fj][:])

    def ev_q(fj, tci, C, ps):
        nc.vector.tensor_scalar_add(
            out=QTt[fj][:, tci * C:(tci + 1) * C], in0=ps[:], scalar1=bq[fj][:])

    projT("wk", xdT, D, T, ev_k)
    projT("wq", xqT, D, TL, ev_q)
    v_proj(xdT, vw, Vst, NKT, vb_bc)
    es_x.close()

    # ---- Phase 2: self-attention ----
    es_ex = ExitStack()
    expp = es_ex.enter_context(tc.tile_pool(name="expp", bufs=12))
    attention(KTt, QTt, Vst, NKT, mergedT, expp, "sa", bounce_idx=0)
    es_ex.close()
    es_kt.close()

    # ---- Phase 3: out-proj + residual + LN1 ----
    es_z1 = ExitStack()
    pz1 = es_z1.enter_context(tc.tile_pool(name="pz1", bufs=1))
    z1 = [pz1.tile([P, TL], F32, tag=f"z1_{k}", name=f"z1_{k}")
          for k in range(KT)]
    z1b = [pz1.tile([P, TL], BF, tag=f"z1b{k}", name=f"z1b{k}")
           for k in range(KT)]

    def ev_o1(fj, tci, C, ps):
        xr = stream2.tile([P, C], F32, tag="xr_s", name="xr_s")
        nc.sync.dma_start(out=xr[:],
                          in_=dram("xres")[fj][:, tci * C:(tci + 1) * C])
        sl = slice(tci * C, (tci + 1) * C)
        nc.vector.scalar_tensor_tensor(
            out=z1[fj][:, sl], in0=ps[:], scalar=bo1[fj][:], in1=xr[:],
            op0=ALU.add, op1=ALU.add)
        nc.vector.tensor_copy(out=z1b[fj][:, sl], in_=z1[fj][:, sl])

    projT("wo1", mergedT, D, TL, ev_o1)
    layer_norm(z1, z1b, g1, be1, out_bf=res_bf,
               spill=[dram("res1_spill")[k] for k in range(KT)])
    es_z1.close()
    es_mg.close()

    # ---- Phase 4: cross QKV ----
    es_pres2 = ExitStack()
    pres2 = es_pres2.enter_context(tc.tile_pool(name="pres2", bufs=1))
    res2_bf = [pres2.tile([P, TL], BF, tag=f"r2b{k}", name=f"r2b{k}")
               for k in range(KT)]

    es_mg2 = ExitStack()
    pmg2 = es_mg2.enter_context(tc.tile_pool(name="pmg2", bufs=1))
    mergedC = [pmg2.tile([P, TL], BF, tag=f"mgc{j}", name=f"mgc{j}")
               for j in range(HP)]

    es_ktc = ExitStack()
    pktc = es_ktc.enter_context(tc.tile_pool(name="pktc", bufs=1))
    KTc = [pktc.tile([P, S], BF, tag=f"KTc{j}", name=f"KTc{j}")
           for j in range(HP)]
    QTc = [pktc.tile([P, TL], BF, tag=f"QTc{j}", name=f"QTc{j}")
           for j in range(HP)]
    Vsc = [pktc.tile([P, H * (HD + 1)], BF, tag=f"Vsc{k}", name=f"Vsc{k}")
           for k in range(NSK)]

    es_xe = ExitStack()
    pxe = es_xe.enter_context(tc.tile_pool(name="pxe", bufs=1))
    xeT, vwc = [], []
    for ki in range(KT):
        tl_ = pxe.tile([P, S], BF, tag=f"xeT{ki}", name=f"xeT{ki}")
        nc.sync.dma_start(out=tl_[:], in_=dram("xeT")[ki])
        xeT.append(tl_)
    for ki in range(KT):
        tl_ = pxe.tile([P, D], BF, tag=f"vwc{ki}", name=f"vwc{ki}")
        nc.sync.dma_start(out=tl_[:], in_=dram("wvc")[ki])
        vwc.append(tl_)

    def ev_kc(fj, tci, C, ps):
        nc.vector.tensor_scalar_add(
            out=KTc[fj][:, tci * C:(tci + 1) * C], in0=ps[:],
            scalar1=bkc[fj][:])

    def ev_qc(fj, tci, C, ps):
        nc.vector.tensor_scalar_add(
            out=QTc[fj][:, tci * C:(tci + 1) * C], in0=ps[:],
            scalar1=bqc[fj][:])

    projT("wkc", xeT, D, S, ev_kc)
    projT("wqc", res_bf, D, TL, ev_qc)
    v_proj(xeT, vwc, Vsc, NSK, vcb_bc)
    es_xe.close()

    # ---- Phase 5: cross-attention ----
    es_ex2 = ExitStack()
    expp2 = es_ex2.enter_context(tc.tile_pool(name="expp2", bufs=12))
    attention(KTc, QTc, Vsc, NSK, mergedC, expp2, "ca", bounce_idx=1)
    es_ex2.close()
    es_ktc.close()

    # ---- Phase 6: cross out-proj + residual + LN2 ----
    es_z2 = ExitStack()
    pz2 = es_z2.enter_context(tc.tile_pool(name="pz2", bufs=1))
    z2 = [pz2.tile([P, TL], F32, tag=f"z2_{k}", name=f"z2_{k}")
          for k in range(KT)]
    z2b = [pz2.tile([P, TL], BF, tag=f"z2b{k}", name=f"z2b{k}")
           for k in range(KT)]

    def ev_o2(fj, tci, C, ps):
        xr = stream2.tile([P, C], F32, tag="xr_s", name="xr_s2")
        nc.sync.dma_start(out=xr[:],
                          in_=dram("res1_spill")[fj][:, tci * C:(tci + 1) * C])
        sl = slice(tci * C, (tci + 1) * C)
        nc.vector.scalar_tensor_tensor(
            out=z2[fj][:, sl], in0=ps[:], scalar=bo2[fj][:], in1=xr[:],
            op0=ALU.add, op1=ALU.add)
        nc.vector.tensor_copy(out=z2b[fj][:, sl], in_=z2[fj][:, sl])

    projT("wo2", mergedC, D, TL, ev_o2)
    layer_norm(z2, z2b, g2, be2, out_bf=res2_bf,
               spill=[dram("res2_spill")[k] for k in range(KT)])
    es_z2.close()
    es_mg2.close()

    # ---- Phase 7: FFN + LN3 -> output ----
    es_ff = ExitStack()
    pff = es_ff.enter_context(tc.tile_pool(name="pff", bufs=1))
    hT = [pff.tile([P, TL], BF, tag=f"hT{f}", name=f"hT{f}")
          for f in range(FT)]

    def ev_f1(fj, tci, C, ps):
        nc.vector.tensor_scalar(
            out=hT[fj][:, tci * C:(tci + 1) * C], in0=ps[:],
            scalar1=b1f[fj][:], scalar2=0.0, op0=ALU.add, op1=ALU.max)

    projT("w1", res2_bf, DFF, TL, ev_f1)

    z3 = [pff.tile([P, TL], F32, tag=f"z3_{k}", name=f"z3_{k}")
          for k in range(KT)]
    z3b = [pff.tile([P, TL], BF, tag=f"z3b{k}", name=f"z3b{k}")
           for k in range(KT)]

    def ev_f2(fj, tci, C, ps):
        xr = stream2.tile([P, C], F32, tag="xr_s", name="xr_s3")
        nc.sync.dma_start(out=xr[:],
                          in_=dram("res2_spill")[fj][:, tci * C:(tci + 1) * C])
        sl = slice(tci * C, (tci + 1) * C)
        nc.vector.scalar_tensor_tensor(
            out=z3[fj][:, sl], in0=ps[:], scalar=b2f[fj][:], in1=xr[:],
            op0=ALU.add, op1=ALU.add)
        nc.vector.tensor_copy(out=z3b[fj][:, sl], in_=z3[fj][:, sl])

    projT("w2", hT, D, TL, ev_f2, kt_in=FT)
    layer_norm(z3, z3b, g3, be3,
               out_dram=[dram("outT")[k] for k in range(KT)])
    es_ff.close()
    es_pres2.close()
    es_pres.close()
    ctx.close()


# ----------------------------------------------------------------------------
# host glue
# ----------------------------------------------------------------------------

def _to_bf(a):
    return np.ascontiguousarray(np.asarray(a).astype(ml_dtypes.bfloat16))


def _to_f32(a):
    return np.ascontiguousarray(np.asarray(a).astype(np.float32))


def _prep_weights(inp, D, H, DFF):
    KT = D // P

    def tile_w(w):  # [Din, F] -> [Din//P, P, F]
        return _to_bf(w.reshape(w.shape[0] // P, P, w.shape[1]))

    hidx = np.arange(H)[:, None] * 3 * HD + np.arange(HD)[None, :]
    perm_q = hidx.ravel()
    perm_k = (hidx + HD).ravel()
    perm_v = (hidx + 2 * HD).ravel()
    qkv_w, qkv_b = inp["qkv_w"], inp["qkv_b"]
    kv_w, kv_b = inp["kv_w"], inp["kv_b"]
    h2 = np.arange(H)[:, None] * 2 * HD + np.arange(HD)[None, :]
    perm_kc = h2.ravel()
    perm_vc = (h2 + HD).ravel()

    def pp(b):  # per-partition bias [F] -> [F//P, P, 1] f32
        return _to_f32(np.asarray(b).reshape(-1, P, 1))

    return dict(
        wq=tile_w(qkv_w[:, perm_q]), wk=tile_w(qkv_w[:, perm_k]),
        wv=tile_w(qkv_w[:, perm_v]),
        bq=pp(qkv_b[perm_q]), bk=pp(qkv_b[perm_k]),
        bv_row=_to_f32(qkv_b[perm_v].reshape(1, D)),
        wo1=tile_w(inp["sa_o_w"]), bo1=pp(inp["sa_o_b"]),
        wqc=tile_w(inp["q_w"]), bqc=pp(inp["q_b"]),
        wkc=tile_w(kv_w[:, perm_kc]), bkc=pp(kv_b[perm_kc]),
        wvc=tile_w(kv_w[:, perm_vc]),
        bvc_row=_to_f32(kv_b[perm_vc].reshape(1, D)),
        wo2=tile_w(inp["ca_o_w"]), bo2=pp(inp["ca_o_b"]),
        w1=tile_w(inp["ff_w1"]), b1f=pp(inp["ff_b1"]),
        w2=tile_w(inp["ff_w2"]), b2f=pp(inp["ff_b2"]),
        g1=pp(inp["g1"]), be1=pp(inp["be1"]),
        g2=pp(inp["g2"]), be2=pp(inp["be2"]),
        g3=pp(inp["g3"]), be3=pp(inp["be3"]),
    )


def make_in_maps(inputs, n_cores=8):
    inp = {k: np.asarray(v) for k, v in inputs.items()}
    B, T, D = inp["x_dec"].shape
    S = inp["x_enc"].shape[1]
    DFF = inp["ff_w1"].shape[1]
    H = D // HD
    KT = D // P
    halves = n_cores // B
    TL = T // halves
    shared = _prep_weights(inp, D, H, DFF)
    in_maps = []
    for c in range(n_cores):
        b, half = c // halves, c % halves
        xd = inp["x_dec"][b]                    # [T, D]
        xe = inp["x_enc"][b]                    # [S, D]
        own = xd[half * TL:(half + 1) * TL]     # [TL, D]
        m = dict(shared)
        m["xdT"] = _to_bf(xd.T.reshape(KT, P, T))
        m["xqT"] = _to_bf(own.T.reshape(KT, P, TL))
        m["xres"] = _to_f32(own.T.reshape(KT, P, TL))
        m["xeT"] = _to_bf(xe.T.reshape(KT, P, S))
        in_maps.append(m)
    return in_maps, (B, T, D, TL, S, DFF, H, halves)


def assemble_output(results, meta):
    B, T, D, TL, S, DFF, H, halves = meta
    out = np.empty((B, T, D), np.float32)
    for c, r in enumerate(results):
        b, half = c // halves, c % halves
        yT = np.asarray(r["outT"]).reshape(D, TL)
        out[b, half * TL:(half + 1) * TL] = yT.T
    return out


def kernel(**inputs):
    in_maps, meta = make_in_maps(inputs)
    B, T, D, TL, S, DFF, H, halves = meta
    nc = build_program(D=D, H=H, T=T, TL=TL, S=S, DFF=DFF)
    res = run_bass_kernel_spmd(nc, in_maps, core_ids=list(range(len(in_maps))))
    return assemble_output(res.results, meta)

